# revision 6
# baseline (speedup 1.0000x reference)
# Trainium2 Bass kernel for nn_DeformableInception (deformable conv x2 -> concat -> 1x1 conv).
#
# Sharding: data-parallel over batch B=8, one sample per NeuronCore (8 cores).
# Weights replicated. No collectives.
#
# Per-core device pipeline (per sample):
#   - x is stored in DRAM as parity-packed row pairs: slot (par, yy, xx) holds
#     image rows (2*yy+par, 2*yy+par+1) x 128ch bf16 (512B). A bilinear 2x2 patch
#     at (yb, xb) is two adjacent slots = ONE contiguous 1KB gather descriptor
#     (>=512B, so no DMA read-modify-write penalty).
#   - per (chunk, branch, tap): SWDGE dma_gather fetches one 1KB patch per output
#     position (positions land on partitions): g[pos, blk, 512] = [v00|v10|v01|v11].
#   - the bilinear blend runs on PE as "diagonal matmuls": for each corner,
#     matmul(out=tp[c, pos], lhsT=g_corner[pos, c], rhs=diag(w_corner)) accumulates
#     the weighted corner into PSUM. The diag tiles (identity * per-position folded
#     corner weight) are built by 4x-mode tensor_scalar on DVE (some on ACT), depend
#     only on host-precomputed weights (not the gather), and are allocated in
#     groups of DIAG_GRP per pool tile to amortize semaphore waits.
#   - tp (f32 PSUM) -> sampT (bf16 SBUF) on ACT, then one PSUM accumulator per
#     chunk takes all 18 taps of both branches: the 1x1 fuse conv and the concat
#     are folded into the per-tap weights on the host (W''_k = W_k @ Wf_br^T),
#     so only a bias-add (ACT activation) and the output DMA remain.
import sys

sys.path.insert(0, "/opt/trn_rl_repo")

import numpy as np
import ml_dtypes

import concourse.bass as bass
import concourse.mybir as mybir
from concourse.tile import TileContext
from concourse.masks import make_identity
from concourse import bacc
from concourse.bass_utils import run_bass_kernel_spmd

bf16 = ml_dtypes.bfloat16

# problem constants (hardcoded per spec)
B = 8
C = 128
H = W = 64
HW = H * W                 # 4096
COUT = 84
K = 3
PAD = 1
KK = K * K                 # 9
NBR = 2                    # two deformable branches
# position chunks; small enough that two PSUM accumulators fit (overlapped
# drain) and the final pipeline drain is short, big enough that gather
# descriptor-prep on Pool stays ahead of the DMA transfers
import os as _osmod
_chunks_env = _osmod.environ.get("KERN_CHUNKS", "2048,2048")
CHUNKS = [int(t) for t in _chunks_env.split(",")]
NCH = len(CHUNKS)
CH_OFF = [sum(CHUNKS[:i]) for i in range(NCH)]
NLISTS = NBR * KK * NCH    # gather lists (one per chunk x branch x tap)
IDXCOLS = HW // 16         # idx cols per (br, tap) across all chunks
WSCOLS = (HW // 128) * 4   # ws cols per (br, tap) across all chunks
NSLOT = 2 * 32 * 64        # 4096 parity-packed patch slots

P = 128
f32 = mybir.dt.float32
bft = mybir.dt.bfloat16
i16 = mybir.dt.int16

import os as _os
# diag builds rotate across DVE / ACT / Pool in a cycle of DIAG_CYCLE:
# first DIAG_POOL_N of each cycle go to Pool (gpsimd), next DIAG_ACT_N to
# ACT, rest to DVE. Pool diag ~273ns, ACT ~292ns, DVE ~94ns in the cost
# model; the split keeps every engine under the DMA-transfer bound.
DIAG_CYCLE = int(_os.environ.get("KERN_DIAG_CYCLE", "16"))
DIAG_POOL_N = int(_os.environ.get("KERN_DIAG_POOL_N", "0"))
DIAG_ACT_N = int(_os.environ.get("KERN_DIAG_ACT_N", "2"))
# for the last TAIL_DVE lists of the run, put ALL runtime diags on DVE —
# ACT (3.1x slower per diag) stays saturated to the end and paces the
# drain, while DVE has slack there
TAIL_DVE = int(_os.environ.get("KERN_TAIL_DVE", "0"))
# tp->sampT copy engine: 1=DVE, 2=ACT (per-copy round robin list)
COPY_ENGS = _os.environ.get("KERN_COPY_ENGS", "2")
GP_BUFS = int(_os.environ.get("KERN_GP_BUFS", "4"))
TPP_BUFS = int(_os.environ.get("KERN_TPP_BUFS", "2"))
SAMP_BUFS = int(_os.environ.get("KERN_SAMP_BUFS", "3"))
DIAG_BUFS = int(_os.environ.get("KERN_DIAG_BUFS", "6"))
DIAG_GRP = int(_os.environ.get("KERN_DIAG_GRP", "16"))  # diags per pool tile
# PSUM budget: out_ps (f32, CHUNKS[0] wide) banks * bufs + tp 2*TPP_BUFS banks
# must fit 8 banks; double-buffer the accumulator only for chunks <= 1024
PREBUILD_LISTS = int(_os.environ.get("KERN_PREBUILD_LISTS", "2"))
FP8_TAPS = int(_os.environ.get("KERN_FP8_TAPS", "1"))
G8_BUFS = int(_os.environ.get("KERN_G8_BUFS", "2"))
POOL_BUILDS = set(int(t) for t in _os.environ.get(
    "KERN_POOL_BUILDS", "").split(",") if t != "")
ACT_BUILDS = set(int(t) for t in _os.environ.get(
    "KERN_ACT_BUILDS", "7,12").split(",") if t != "")
FP8_KSET = set(int(t) for t in
               _os.environ.get("KERN_FP8_KSET", "1,4").split(","))
BIGP_BUFS = int(_os.environ.get("KERN_BIGP_BUFS", "0"))
if BIGP_BUFS == 0:
    BIGP_BUFS = 2 if CHUNKS[0] <= 1024 else 1

_CACHE = {}


def _host_precompute(x, dm0, dm1, w0, w1, wf, bfv):
    """Numpy precompute: patch-slot gather indices + 2D-folded bilinear corner
    weights, parity-packed x, weight repacks."""
    ky = np.repeat(np.arange(K) - PAD, K).astype(np.float32)
    kx = np.tile(np.arange(K) - PAD, K).astype(np.float32)
    base_y = np.arange(H, dtype=np.float32).reshape(1, 1, H, 1)
    base_x = np.arange(W, dtype=np.float32).reshape(1, 1, 1, W)

    idx_all = np.zeros((B, NBR, KK, HW), np.int16)        # patch slot per (tap,pos)
    w_all = np.zeros((B, NBR, KK, 4, HW), np.float32)     # r0c0,r1c0,r0c1,r1c1

    for br, dm in ((0, dm0), (1, dm1)):
        off = dm.reshape(B, KK, 2, H, W)
        py = off[:, :, 0] + base_y + ky.reshape(1, KK, 1, 1)
        px = off[:, :, 1] + base_x + kx.reshape(1, KK, 1, 1)
        y0 = np.floor(py); x0 = np.floor(px)
        wy1 = py - y0; wx1 = px - x0
        wy0 = 1.0 - wy1; wx0 = 1.0 - wx1
        y0i = y0.astype(np.int64); x0i = x0.astype(np.int64)
        yb = np.clip(y0i, 0, H - 2)
        xb = np.clip(x0i, 0, W - 2)
        slot = (yb & 1) * (32 * 64) + (yb >> 1) * 64 + xb
        idx_all[:, br] = slot.reshape(B, KK, HW).astype(np.int16)
        w4 = np.zeros((2, 2) + py.shape, np.float32)      # [rp, cp, B, KK, H, W]
        for r, wy in ((0, wy0), (1, wy1)):
            yi = y0i + r
            rv = ((yi >= 0) & (yi < H)).astype(np.float32)
            rp = np.clip(yi, 0, H - 1) - yb               # 0 or 1
            for c, wx in ((0, wx0), (1, wx1)):
                xi = x0i + c
                cv = ((xi >= 0) & (xi < W)).astype(np.float32)
                cp = np.clip(xi, 0, W - 1) - xb
                contrib = wy * wx * rv * cv
                for rr in (0, 1):
                    for cc in (0, 1):
                        w4[rr, cc] += np.where((rp == rr) & (cp == cc), contrib, 0.0)
        # corner order matches patch byte layout [v00, v10, v01, v11]
        w_all[:, br, :, 0] = w4[0, 0].reshape(B, KK, HW)
        w_all[:, br, :, 1] = w4[1, 0].reshape(B, KK, HW)
        w_all[:, br, :, 2] = w4[0, 1].reshape(B, KK, HW)
        w_all[:, br, :, 3] = w4[1, 1].reshape(B, KK, HW)

    # xPP [B, NSLOT+2, 2C] bf16: slot (par, yy, xx) = rows (2yy+par, 2yy+par+1)
    xhwc = np.transpose(x, (0, 2, 3, 1))                  # [B, H, W, C]
    xPP = np.zeros((B, 2, 32, 64, 2, C), np.float32)
    for par in (0, 1):
        for rp in (0, 1):
            start = par + rp
            rows = xhwc[:, start::2, :, :]                # [B, n, W, C]
            n = min(rows.shape[1], 32)
            xPP[:, par, :n, :, rp, :] = rows[:, :n]
    xPP = xPP.reshape(B, NSLOT, 2 * C)
    xPPf = np.concatenate([xPP, np.zeros((B, 2, 2 * C), np.float32)], axis=1)
    xPPp = xPPf.astype(bf16)
    xPP8 = xPPf.astype(ml_dtypes.float8_e4m3fn)

    # IDX [B, 128, totalcols] int16: lists in consumption order (chunk, br, k),
    # each wrapped (j%16, j//16), replicated across the 8 gpsimd cores'
    # 16-partition groups, then concatenated along cols so one DMA loads it.
    # WS [B, 128, totalwcols] f32 likewise (per block: 4 corner weights).
    idx_cols = []
    ws_cols = []
    for ci in range(NCH):
        off, sz = CH_OFF[ci], CHUNKS[ci]
        for br in range(NBR):
            for k in range(KK):
                seq = idx_all[:, br, k, off:off + sz]     # [B, sz]
                wr = np.transpose(seq.reshape(B, sz // 16, 16), (0, 2, 1))
                wr = np.broadcast_to(wr[:, None, :, :], (B, 8, 16, sz // 16))
                idx_cols.append(wr.reshape(B, P, sz // 16))
                wsl = w_all[:, br, k, :, off:off + sz]    # [B, 4, sz]
                wsl = wsl.reshape(B, 4, sz // 128, P)
                wsl = np.transpose(wsl, (0, 3, 2, 1))     # [B, p, blk, c4]
                ws_cols.append(wsl.reshape(B, P, (sz // 128) * 4))
    IDX = np.ascontiguousarray(np.concatenate(idx_cols, axis=2))
    WS = np.ascontiguousarray(np.concatenate(ws_cols, axis=2)).astype(bf16)

    # W0T [128, NTAPS*COUT] bf16: lhsT per (branch, tap) with the 1x1 fuse conv
    # folded in (W''_k = W_k @ Wf_br^T), device layout
    WFT = [wf[:, :COUT, 0, 0].T, wf[:, COUT:, 0, 0].T]    # [84in, 84out] per br
    W0T = np.zeros((NBR * KK, C, COUT), np.float32)
    for br, w in ((0, w0), (1, w1)):
        for k in range(KK):
            W0T[br * KK + k] = w[:, :, k // K, k % K].T @ WFT[br]
    W0T = np.ascontiguousarray(np.transpose(W0T, (1, 0, 2))).astype(bf16)

    BF = bfv.reshape(COUT, 1).astype(np.float32)
    return xPPp, xPP8, IDX, WS, W0T, BF


def _build_nc():
    nc = bacc.Bacc()
    xpp_d = nc.declare_dram_parameter("xpp", [NSLOT + 2, 2 * C], bft, isOutput=False)
    xpp8_d = nc.declare_dram_parameter("xpp8", [NSLOT + 2, 2 * C],
                                       mybir.dt.float8e4, isOutput=False)
    idx_d = nc.declare_dram_parameter("idx", [P, NBR * KK * IDXCOLS], i16, isOutput=False)
    ws_d = nc.declare_dram_parameter("ws", [P, NBR * KK * WSCOLS], bft, isOutput=False)
    w0_d = nc.declare_dram_parameter("w0t", [C, NBR * KK * COUT], bft, isOutput=False)
    bf_d = nc.declare_dram_parameter("bfv", [COUT, 1], f32, isOutput=False)
    out_d = nc.declare_dram_parameter("out", [COUT, HW], bft, isOutput=True)

    # patch gather source: elem i = slot i (256 elems), read 512 elems (2 slots)
    src_ap = bass.AP(tensor=xpp_d, offset=0, ap=[[2 * C, NSLOT], [1, 4 * C]])
    src_ap8 = bass.AP(tensor=xpp8_d, offset=0, ap=[[2 * C, NSLOT], [1, 4 * C]])

    copy_engs = [int(t) for t in COPY_ENGS.split(",")]

    with TileContext(nc) as tc:
        with tc.tile_pool(name="const", bufs=1) as const, \
             tc.tile_pool(name="gp", bufs=GP_BUFS) as gp, \
             tc.tile_pool(name="glp", bufs=1) as glp, \
             tc.tile_pool(name="g8p", bufs=G8_BUFS) as g8p, \
             tc.tile_pool(name="dgp", bufs=DIAG_BUFS) as dgp, \
             tc.tile_pool(name="sampp", bufs=SAMP_BUFS) as sampp, \
             tc.tile_pool(name="op", bufs=2) as op, \
             tc.tile_pool(name="tpp", bufs=TPP_BUFS, space="PSUM") as tpp, \
             tc.tile_pool(name="bigp", bufs=BIGP_BUFS, space="PSUM") as bigp:
            TIC = NBR * KK * IDXCOLS
            TWC = NBR * KK * WSCOLS
            # split the idx/ws loads so the first gathers launch ASAP;
            # issue them before any compute setup so the DMA queue drains
            # while identity/ws-convert run
            NIH = 2 * (CHUNKS[0] // 16)
            NWH = 2 * ((CHUNKS[0] // 128) * 4)
            idx_t = const.tile([P, TIC], i16)
            nc.sync.dma_start(out=idx_t[:, 0:NIH], in_=idx_d[:, 0:NIH])
            ws_b = const.tile([P, TWC], bft)
            nc.sync.dma_start(out=ws_b[:, 0:NWH], in_=ws_d[:, 0:NWH])
            ident = const.tile([P, P], bft)
            make_identity(nc, ident[:])
            ws_t = const.tile([P, TWC], f32)
            nc.vector.tensor_copy(out=ws_t[:, 0:NWH], in_=ws_b[:, 0:NWH])
            nc.sync.dma_start(out=idx_t[:, NIH:], in_=idx_d[:, NIH:TIC])
            nc.sync.dma_start(out=ws_b[:, NWH:], in_=ws_d[:, NWH:TWC])
            nc.vector.tensor_copy(out=ws_t[:, NWH:], in_=ws_b[:, NWH:])
            w0_t = const.tile([C, NBR * KK, COUT], bft)
            nc.sync.dma_start(out=w0_t[:], in_=w0_d[:])
            bf_t = const.tile([COUT, 1], f32)
            nc.sync.dma_start(out=bf_t[:], in_=bf_d[:])

            # prebuild the final list's diag tiles during the idle startup
            # window (they depend only on ws), so the drain isn't gated on
            # the loaded diag-build pipeline
            nlast = (CHUNKS[-1] // 128) * 4 * PREBUILD_LISTS
            dlast = const.tile([P, nlast, P], bft)
            wo_last = TWC - nlast
            for di in range(nlast):
                nc.vector.tensor_scalar(
                    out=dlast[:, di, :], in0=ident[:],
                    scalar1=ws_t[:, wo_last + di:wo_last + di + 1],
                    scalar2=None, op0=mybir.AluOpType.mult,
                )

            ndiag = 0
            ncopy = 0
            io, wo = 0, 0
            dgrp = None
            for ci in range(NCH):
                off, sz = CH_OFF[ci], CHUNKS[ci]
                nb = sz // 128
                out_ps = bigp.tile([COUT, CHUNKS[0]], f32, tag="big")
                for br in range(NBR):
                    for k in range(KK):
                        t = br * KK + k
                        last = (ci == NCH - 1 and br == NBR - 1
                                and k == KK - 1 and sz >= 1024)
                        tail2 = False
                        use8 = (FP8_TAPS and k in FP8_KSET and not last)
                        if last:
                            # split the final gather into separate tiles so
                            # earlier strips process while later parts
                            # transfer; asymmetric so only the final 512
                            # positions depend on the very last transfer
                            import os as _o
                            _gs = _o.environ.get("KERN_GSPLIT", "")
                            if _gs:
                                gsplit = [int(x) for x in _gs.split(",")]
                            elif sz == 2048:
                                gsplit = [1024, 512, 512]
                            else:
                                gsplit = [sz // 2, sz // 2]
                            ghalves = []
                            goff = 0
                            for gh, hsz in enumerate(gsplit):
                                gt = glp.tile([P, hsz // 128, 4 * C], bft,
                                              tag=f"glast{gh}")
                                nc.gpsimd.dma_gather(
                                    out_ap=gt[:, 0:hsz // 128, :],
                                    in_ap=src_ap,
                                    idxs_ap=idx_t[:, io + goff // 16:
                                                  io + (goff + hsz) // 16],
                                    num_idxs=hsz, num_idxs_reg=hsz,
                                    elem_size=4 * C, elem_step=2 * C,
                                    transpose=False, single_packet=False,
                                )
                                ghalves.append((goff // 128, gt))
                                goff += hsz
                            g = None
                        elif use8:
                            g = g8p.tile([P, CHUNKS[0] // 128, 4 * C],
                                         mybir.dt.float8e4, tag="g8")
                            nc.gpsimd.dma_gather(
                                out_ap=g[:, 0:nb, :], in_ap=src_ap8,
                                idxs_ap=idx_t[:, io:io + sz // 16],
                                num_idxs=sz, num_idxs_reg=sz,
                                elem_size=4 * C, elem_step=2 * C, transpose=False,
                                single_packet=False,
                            )
                        else:
                            g = gp.tile([P, CHUNKS[0] // 128, 4 * C], bft,
                                        tag="g")
                            nc.gpsimd.dma_gather(
                                out_ap=g[:, 0:nb, :], in_ap=src_ap,
                                idxs_ap=idx_t[:, io:io + sz // 16],
                                num_idxs=sz, num_idxs_reg=sz,
                                elem_size=4 * C, elem_step=2 * C, transpose=False,
                                single_packet=False,
                            )
                        sampT = sampp.tile([C, CHUNKS[0]], bft, tag="sampT")
                        if last:
                            ob_last = op.tile([COUT, CHUNKS[0]], bft,
                                              tag="outsb")
                        if last or tail2:
                            strips = [(i * 4, 4) for i in range(nb // 4)]
                        else:
                            strips = [(i * 8, min(8, nb - i * 8))
                                      for i in range((nb + 7) // 8)]
                        for qh, (qb0, qnb) in enumerate(strips):
                            tp = tpp.tile([C, 1024], f32, tag="tp")
                            for jb in range(qnb):
                                b = qb0 + jb
                                for c4 in range(4):
                                    if (ci == NCH - 1 and br == NBR - 1
                                            and k >= KK - PREBUILD_LISTS):
                                        di = ((k - (KK - PREBUILD_LISTS))
                                              * (CHUNKS[-1] // 128) * 4
                                              + b * 4 + c4)
                                        diag = dlast[:, di, :]
                                    else:
                                        gi = ndiag % DIAG_GRP
                                        if gi == 0:
                                            dgrp = dgp.tile([P, DIAG_GRP, P],
                                                            bft, tag="diag")
                                        diag = dgrp[:, gi, :]
                                        sc = ws_t[:, wo + b * 4 + c4:
                                                  wo + b * 4 + c4 + 1]
                                        r = ndiag % DIAG_CYCLE
                                        ndiag += 1
                                        listpos = ci * NBR * KK + br * KK + k
                                        if listpos >= NCH * NBR * KK - TAIL_DVE:
                                            r = DIAG_CYCLE - 1  # force DVE
                                        if r < DIAG_POOL_N:
                                            nc.gpsimd.tensor_scalar(
                                                out=diag, in0=ident[:],
                                                scalar1=sc, scalar2=None,
                                                op0=mybir.AluOpType.mult,
                                            )
                                        elif r < DIAG_POOL_N + DIAG_ACT_N:
                                            nc.scalar.activation(
                                                out=diag, in_=ident[:],
                                                func=mybir.ActivationFunctionType.Identity,
                                                scale=sc,
                                            )
                                        else:
                                            nc.vector.tensor_scalar(
                                                out=diag, in0=ident[:],
                                                scalar1=sc, scalar2=None,
                                                op0=mybir.AluOpType.mult,
                                            )
                                    if last:
                                        for gi_, gt_ in reversed(ghalves):
                                            if b >= gi_:
                                                gsrc = gt_[:, b - gi_, :]
                                                break
                                    else:
                                        gsrc = g[:, b, :]
                                    nc.tensor.matmul(
                                        out=tp[:, jb * P:(jb + 1) * P],
                                        lhsT=gsrc[:, c4 * C:(c4 + 1) * C],
                                        rhs=diag,
                                        start=(c4 == 0), stop=(c4 == 3),
                                    )
                            qoff = qb0 * P
                            dst = sampT[:, qoff:qoff + qnb * P]
                            ce = 1 if last else copy_engs[ncopy % len(copy_engs)]
                            ncopy += 1
                            if ce == 1:
                                nc.vector.tensor_copy(out=dst,
                                                      in_=tp[:, 0:qnb * P])
                            else:
                                nc.scalar.copy(out=dst, in_=tp[:, 0:qnb * P])
                            if last or tail2:
                                # deform per 512-strip so the drain pipelines
                                nc.tensor.matmul(
                                    out=out_ps[:, qoff:qoff + qnb * P],
                                    lhsT=w0_t[:, t, :],
                                    rhs=sampT[:, qoff:qoff + qnb * P],
                                    start=False, stop=last,
                                )
                                if last:
                                    nc.scalar.activation(
                                        out=ob_last[:, qoff:qoff + qnb * P],
                                        in_=out_ps[:, qoff:qoff + qnb * P],
                                        func=mybir.ActivationFunctionType.Identity,
                                        bias=bf_t[:], scale=1.0,
                                    )
                                    nc.sync.dma_start(
                                        out=out_d[:, off + qoff:
                                                  off + qoff + qnb * P],
                                        in_=ob_last[:, qoff:qoff + qnb * P])
                        if not (last or tail2):
                            for cs in range(0, sz, 512):
                                ce_ = min(cs + 512, sz)
                                nc.tensor.matmul(
                                    out=out_ps[:, cs:ce_],
                                    lhsT=w0_t[:, t, :],
                                    rhs=sampT[:, cs:ce_],
                                    start=(br == 0 and k == 0),
                                    stop=(br == NBR - 1 and k == KK - 1),
                                )
                        io += sz // 16
                        wo += (sz // 128) * 4
                if ci == NCH - 1 and sz >= 1024:
                    pass  # drained per strip inside the last list's loop
                else:
                    out_sb = op.tile([COUT, CHUNKS[0]], bft, tag="outsb")
                    nc.scalar.activation(
                        out=out_sb[:, 0:sz], in_=out_ps[:, 0:sz],
                        func=mybir.ActivationFunctionType.Identity, bias=bf_t[:],
                        scale=1.0,
                    )
                    nc.sync.dma_start(out=out_d[:, off:off + sz],
                                      in_=out_sb[:, 0:sz])
    nc.finalize()
    return nc


def kernel(x, dm0, dm1, w0, w1, wf, bf):
    x = np.asarray(x, np.float32)
    dm0 = np.asarray(dm0, np.float32)
    dm1 = np.asarray(dm1, np.float32)
    w0 = np.asarray(w0, np.float32)
    w1 = np.asarray(w1, np.float32)
    wf = np.asarray(wf, np.float32)
    bfv = np.asarray(bf, np.float32)

    xPPp, xPP8, IDX, WS, W0T, BF = _host_precompute(x, dm0, dm1, w0, w1, wf, bfv)

    if "nc" not in _CACHE:
        _CACHE["nc"] = _build_nc()
    nc = _CACHE["nc"]

    in_maps = [
        {
            "xpp": np.ascontiguousarray(xPPp[i]),
            "xpp8": np.ascontiguousarray(xPP8[i]),
            "idx": np.ascontiguousarray(IDX[i].reshape(P, -1)),
            "ws": np.ascontiguousarray(WS[i].reshape(P, -1)),
            "w0t": W0T.reshape(C, -1),
            "bfv": BF,
        }
        for i in range(B)
    ]
    res = run_bass_kernel_spmd(nc, in_maps, core_ids=list(range(B)),
                               **_CACHE.get("run_kwargs", {}))
    _CACHE["last_results"] = res
    out = np.stack([np.asarray(res.results[i]["out"], np.float32)
                    for i in range(B)])
    return out.reshape(B, COUT, H, W)



# revision 22
# speedup vs baseline: 1.0498x; 1.0498x over previous
# Trainium2 Bass kernel for nn_DeformableInception (deformable conv x2 -> concat -> 1x1 conv).
#
# Sharding: data-parallel over batch B=8, one sample per NeuronCore (8 cores).
# Weights replicated. No collectives.
#
# Per-core device pipeline (per sample):
#   - x is stored in DRAM as parity-packed row pairs: slot (par, yy, xx) holds
#     image rows (2*yy+par, 2*yy+par+1) x 128ch bf16 (512B). A bilinear 2x2 patch
#     at (yb, xb) is two adjacent slots = ONE contiguous 1KB gather descriptor
#     (>=512B, so no DMA read-modify-write penalty).
#   - per (chunk, branch, tap): SWDGE dma_gather fetches one 1KB patch per output
#     position (positions land on partitions): g[pos, blk, 512] = [v00|v10|v01|v11].
#   - the bilinear blend runs on PE as "diagonal matmuls": for each corner,
#     matmul(out=tp[c, pos], lhsT=g_corner[pos, c], rhs=diag(w_corner)) accumulates
#     the weighted corner into PSUM. The diag tiles (identity * per-position folded
#     corner weight) are built by 4x-mode tensor_scalar on DVE (some on ACT), depend
#     only on host-precomputed weights (not the gather), and are allocated in
#     groups of DIAG_GRP per pool tile to amortize semaphore waits.
#   - tp (f32 PSUM) -> sampT (bf16 SBUF) on ACT, then one PSUM accumulator per
#     chunk takes all 18 taps of both branches: the 1x1 fuse conv and the concat
#     are folded into the per-tap weights on the host (W''_k = W_k @ Wf_br^T),
#     so only a bias-add (ACT activation) and the output DMA remain.
import sys

sys.path.insert(0, "/opt/trn_rl_repo")

import numpy as np
import ml_dtypes

import concourse.bass as bass
import concourse.mybir as mybir
from concourse.tile import TileContext
from concourse.masks import make_identity
from concourse import bacc
from concourse.bass_utils import run_bass_kernel_spmd

bf16 = ml_dtypes.bfloat16

# problem constants (hardcoded per spec)
B = 8
C = 128
H = W = 64
HW = H * W                 # 4096
COUT = 84
K = 3
PAD = 1
KK = K * K                 # 9
NBR = 2                    # two deformable branches
# position chunks; small enough that two PSUM accumulators fit (overlapped
# drain) and the final pipeline drain is short, big enough that gather
# descriptor-prep on Pool stays ahead of the DMA transfers
import os as _osmod
_chunks_env = _osmod.environ.get("KERN_CHUNKS", "2048,2048")
CHUNKS = [int(t) for t in _chunks_env.split(",")]
NCH = len(CHUNKS)
CH_OFF = [sum(CHUNKS[:i]) for i in range(NCH)]
NLISTS = NBR * KK * NCH    # gather lists (one per chunk x branch x tap)
IDXCOLS = HW // 16         # idx cols per (br, tap) across all chunks
WSCOLS = (HW // 128) * 4   # ws cols per (br, tap) across all chunks
NSLOT = 2 * 32 * 64        # 4096 parity-packed patch slots

P = 128
f32 = mybir.dt.float32
bft = mybir.dt.bfloat16
i16 = mybir.dt.int16

import os as _os
# diag builds rotate across DVE / ACT / Pool in a cycle of DIAG_CYCLE:
# first DIAG_POOL_N of each cycle go to Pool (gpsimd), next DIAG_ACT_N to
# ACT, rest to DVE. Pool diag ~273ns, ACT ~292ns, DVE ~94ns in the cost
# model; the split keeps every engine under the DMA-transfer bound.
DIAG_CYCLE = int(_os.environ.get("KERN_DIAG_CYCLE", "16"))
DIAG_POOL_N = int(_os.environ.get("KERN_DIAG_POOL_N", "0"))
DIAG_ACT_N = int(_os.environ.get("KERN_DIAG_ACT_N", "2"))
# for the last TAIL_DVE lists of the run, put ALL runtime diags on DVE —
# ACT (3.1x slower per diag) stays saturated to the end and paces the
# drain, while DVE has slack there
TAIL_DVE = int(_os.environ.get("KERN_TAIL_DVE", "8"))
# within the tail, send every TAIL_POOL_N-of-4 diags to Pool (its SEQ is
# free of gather waits once the last gathers have issued)
TAIL_POOL_N = int(_os.environ.get("KERN_TAIL_POOL_N", "0"))
# tp->sampT copy engine: 1=DVE, 2=ACT (per-copy round robin list)
COPY_ENGS = _os.environ.get("KERN_COPY_ENGS", "2")
GP_BUFS = int(_os.environ.get("KERN_GP_BUFS", "4"))
TPP_BUFS = int(_os.environ.get("KERN_TPP_BUFS", "2"))
SAMP_BUFS = int(_os.environ.get("KERN_SAMP_BUFS", "3"))
DIAG_BUFS = int(_os.environ.get("KERN_DIAG_BUFS", "10"))
DIAG_A_BUFS = int(_os.environ.get("KERN_DIAG_A_BUFS", "3"))
DIAG_GRP = int(_os.environ.get("KERN_DIAG_GRP", "16"))  # diags per pool tile
# PSUM budget: out_ps (f32, CHUNKS[0] wide) banks * bufs + tp 2*TPP_BUFS banks
# must fit 8 banks; double-buffer the accumulator only for chunks <= 1024
PREBUILD_LISTS = int(_os.environ.get("KERN_PREBUILD_LISTS", "2"))
FP8_TAPS = int(_os.environ.get("KERN_FP8_TAPS", "1"))
G8_BUFS = int(_os.environ.get("KERN_G8_BUFS", "2"))
POOL_BUILDS = set(int(t) for t in _os.environ.get(
    "KERN_POOL_BUILDS", "").split(",") if t != "")
ACT_BUILDS = set(int(t) for t in _os.environ.get(
    "KERN_ACT_BUILDS", "7,12").split(",") if t != "")
FP8_KSET = set(int(t) for t in
               _os.environ.get("KERN_FP8_KSET", "1,4").split(","))
BIGP_BUFS = int(_os.environ.get("KERN_BIGP_BUFS", "0"))
if BIGP_BUFS == 0:
    BIGP_BUFS = 2 if CHUNKS[0] <= 1024 else 1

_CACHE = {}


def _host_precompute(x, dm0, dm1, w0, w1, wf, bfv):
    """Numpy precompute: patch-slot gather indices + 2D-folded bilinear corner
    weights, parity-packed x, weight repacks."""
    ky = np.repeat(np.arange(K) - PAD, K).astype(np.float32)
    kx = np.tile(np.arange(K) - PAD, K).astype(np.float32)
    base_y = np.arange(H, dtype=np.float32).reshape(1, 1, H, 1)
    base_x = np.arange(W, dtype=np.float32).reshape(1, 1, 1, W)

    idx_all = np.zeros((B, NBR, KK, HW), np.int16)        # patch slot per (tap,pos)
    w_all = np.zeros((B, NBR, KK, 4, HW), np.float32)     # r0c0,r1c0,r0c1,r1c1

    for br, dm in ((0, dm0), (1, dm1)):
        off = dm.reshape(B, KK, 2, H, W)
        py = off[:, :, 0] + base_y + ky.reshape(1, KK, 1, 1)
        px = off[:, :, 1] + base_x + kx.reshape(1, KK, 1, 1)
        y0 = np.floor(py); x0 = np.floor(px)
        wy1 = py - y0; wx1 = px - x0
        wy0 = 1.0 - wy1; wx0 = 1.0 - wx1
        y0i = y0.astype(np.int64); x0i = x0.astype(np.int64)
        yb = np.clip(y0i, 0, H - 2)
        xb = np.clip(x0i, 0, W - 2)
        slot = (yb & 1) * (32 * 64) + (yb >> 1) * 64 + xb
        idx_all[:, br] = slot.reshape(B, KK, HW).astype(np.int16)
        w4 = np.zeros((2, 2) + py.shape, np.float32)      # [rp, cp, B, KK, H, W]
        for r, wy in ((0, wy0), (1, wy1)):
            yi = y0i + r
            rv = ((yi >= 0) & (yi < H)).astype(np.float32)
            rp = np.clip(yi, 0, H - 1) - yb               # 0 or 1
            for c, wx in ((0, wx0), (1, wx1)):
                xi = x0i + c
                cv = ((xi >= 0) & (xi < W)).astype(np.float32)
                cp = np.clip(xi, 0, W - 1) - xb
                contrib = wy * wx * rv * cv
                for rr in (0, 1):
                    for cc in (0, 1):
                        w4[rr, cc] += np.where((rp == rr) & (cp == cc), contrib, 0.0)
        # corner order matches patch byte layout [v00, v10, v01, v11]
        w_all[:, br, :, 0] = w4[0, 0].reshape(B, KK, HW)
        w_all[:, br, :, 1] = w4[1, 0].reshape(B, KK, HW)
        w_all[:, br, :, 2] = w4[0, 1].reshape(B, KK, HW)
        w_all[:, br, :, 3] = w4[1, 1].reshape(B, KK, HW)

    # xPP [B, NSLOT+2, 2C] bf16: slot (par, yy, xx) = rows (2yy+par, 2yy+par+1)
    xhwc = np.transpose(x, (0, 2, 3, 1))                  # [B, H, W, C]
    xPP = np.zeros((B, 2, 32, 64, 2, C), np.float32)
    for par in (0, 1):
        for rp in (0, 1):
            start = par + rp
            rows = xhwc[:, start::2, :, :]                # [B, n, W, C]
            n = min(rows.shape[1], 32)
            xPP[:, par, :n, :, rp, :] = rows[:, :n]
    xPP = xPP.reshape(B, NSLOT, 2 * C)
    xPPf = np.concatenate([xPP, np.zeros((B, 2, 2 * C), np.float32)], axis=1)
    xPPp = xPPf.astype(bf16)
    xPP8 = xPPf.astype(ml_dtypes.float8_e4m3fn)

    # IDX [B, 128, totalcols] int16: lists in consumption order (chunk, br, k),
    # each wrapped (j%16, j//16), replicated across the 8 gpsimd cores'
    # 16-partition groups, then concatenated along cols so one DMA loads it.
    # WS [B, 128, totalwcols] f32 likewise (per block: 4 corner weights).
    idx_cols = []
    ws_cols = []
    for ci in range(NCH):
        off, sz = CH_OFF[ci], CHUNKS[ci]
        for br in range(NBR):
            for k in range(KK):
                seq = idx_all[:, br, k, off:off + sz]     # [B, sz]
                wr = np.transpose(seq.reshape(B, sz // 16, 16), (0, 2, 1))
                wr = np.broadcast_to(wr[:, None, :, :], (B, 8, 16, sz // 16))
                idx_cols.append(wr.reshape(B, P, sz // 16))
                wsl = w_all[:, br, k, :, off:off + sz]    # [B, 4, sz]
                wsl = wsl.reshape(B, 4, sz // 128, P)
                wsl = np.transpose(wsl, (0, 3, 2, 1))     # [B, p, blk, c4]
                ws_cols.append(wsl.reshape(B, P, (sz // 128) * 4))
    IDX = np.ascontiguousarray(np.concatenate(idx_cols, axis=2))
    WS = np.ascontiguousarray(np.concatenate(ws_cols, axis=2)).astype(bf16)

    # W0T [128, NTAPS*COUT] bf16: lhsT per (branch, tap) with the 1x1 fuse conv
    # folded in (W''_k = W_k @ Wf_br^T), device layout
    WFT = [wf[:, :COUT, 0, 0].T, wf[:, COUT:, 0, 0].T]    # [84in, 84out] per br
    W0T = np.zeros((NBR * KK, C, COUT), np.float32)
    for br, w in ((0, w0), (1, w1)):
        for k in range(KK):
            W0T[br * KK + k] = w[:, :, k // K, k % K].T @ WFT[br]
    W0T = np.ascontiguousarray(np.transpose(W0T, (1, 0, 2))).astype(bf16)

    BF = bfv.reshape(COUT, 1).astype(np.float32)
    return xPPp, xPP8, IDX, WS, W0T, BF


def _build_nc():
    nc = bacc.Bacc()
    xpp_d = nc.declare_dram_parameter("xpp", [NSLOT + 2, 2 * C], bft, isOutput=False)
    xpp8_d = nc.declare_dram_parameter("xpp8", [NSLOT + 2, 2 * C],
                                       mybir.dt.float8e4, isOutput=False)
    idx_d = nc.declare_dram_parameter("idx", [P, NBR * KK * IDXCOLS], i16, isOutput=False)
    ws_d = nc.declare_dram_parameter("ws", [P, NBR * KK * WSCOLS], bft, isOutput=False)
    w0_d = nc.declare_dram_parameter("w0t", [C, NBR * KK * COUT], bft, isOutput=False)
    bf_d = nc.declare_dram_parameter("bfv", [COUT, 1], f32, isOutput=False)
    out_d = nc.declare_dram_parameter("out", [COUT, HW], bft, isOutput=True)

    # patch gather source: elem i = slot i (256 elems), read 512 elems (2 slots)
    src_ap = bass.AP(tensor=xpp_d, offset=0, ap=[[2 * C, NSLOT], [1, 4 * C]])
    src_ap8 = bass.AP(tensor=xpp8_d, offset=0, ap=[[2 * C, NSLOT], [1, 4 * C]])

    copy_engs = [int(t) for t in COPY_ENGS.split(",")]

    with TileContext(nc) as tc:
        with tc.tile_pool(name="const", bufs=1) as const, \
             tc.tile_pool(name="gp", bufs=GP_BUFS) as gp, \
             tc.tile_pool(name="glp", bufs=1) as glp, \
             tc.tile_pool(name="g8p", bufs=G8_BUFS) as g8p, \
             tc.tile_pool(name="dgp", bufs=DIAG_BUFS) as dgp, \
             tc.tile_pool(name="dgpa", bufs=DIAG_A_BUFS) as dgpa, \
             tc.tile_pool(name="sampp", bufs=SAMP_BUFS) as sampp, \
             tc.tile_pool(name="op", bufs=2) as op, \
             tc.tile_pool(name="tpp", bufs=TPP_BUFS, space="PSUM") as tpp, \
             tc.tile_pool(name="bigp", bufs=BIGP_BUFS, space="PSUM") as bigp:
            TIC = NBR * KK * IDXCOLS
            TWC = NBR * KK * WSCOLS
            # split the idx/ws loads so the first gathers launch ASAP;
            # issue them before any compute setup so the DMA queue drains
            # while identity/ws-convert run
            NIH = 2 * (CHUNKS[0] // 16)
            NWH = 2 * ((CHUNKS[0] // 128) * 4)
            idx_t = const.tile([P, TIC], i16)
            nc.sync.dma_start(out=idx_t[:, 0:NIH], in_=idx_d[:, 0:NIH])
            ws_b = const.tile([P, TWC], bft)
            nc.sync.dma_start(out=ws_b[:, 0:NWH], in_=ws_d[:, 0:NWH])
            ident = const.tile([P, P], bft)
            make_identity(nc, ident[:])
            ws_t = const.tile([P, TWC], f32)
            nc.scalar.copy(out=ws_t[:, 0:NWH], in_=ws_b[:, 0:NWH])
            nc.sync.dma_start(out=idx_t[:, NIH:], in_=idx_d[:, NIH:TIC])
            nc.sync.dma_start(out=ws_b[:, NWH:], in_=ws_d[:, NWH:TWC])
            nc.scalar.copy(out=ws_t[:, NWH:], in_=ws_b[:, NWH:])
            w0_t = const.tile([C, NBR * KK, COUT], bft)
            nc.sync.dma_start(out=w0_t[:], in_=w0_d[:])
            bf_t = const.tile([COUT, 1], f32)
            nc.sync.dma_start(out=bf_t[:], in_=bf_d[:])

            # --- software-pipelined diag builds -------------------------
            # Diags depend only on ws, so build list L's diags during the
            # processing of list L-1 (one full DMA period of slack). This
            # removes diag-build latency from the per-list critical chain:
            # blends never wait on a diag that was emitted just before them.
            NLISTS_T = NCH * NBR * KK
            wo_of = [0]
            for ci_ in range(NCH):
                for _ in range(NBR * KK):
                    wo_of.append(wo_of[-1] + (CHUNKS[ci_] // 128) * 4)
            diag_store = {}  # L -> list of 64 diag APs
            # per-engine group tiles so one engine's SEQ never blocks on a
            # buffer whose writers/consumers belong to the other engine
            state = {"nD": 0, "gD": None, "nA": 0, "gA": None, "nsel": 0}

            def build_diags(L):
                nd = (CHUNKS[L // (NBR * KK)] // 128) * 4
                wo_ = wo_of[L]
                aps = []
                for di in range(nd):
                    sc = ws_t[:, wo_ + di:wo_ + di + 1]
                    r = state["nsel"] % DIAG_CYCLE
                    if L >= NLISTS_T - TAIL_DVE:
                        # drain: no ACT diags; optionally rotate some to Pool
                        if TAIL_POOL_N and state["nsel"] % 4 < TAIL_POOL_N:
                            eng = "P"
                        else:
                            eng = "D"
                    elif r < DIAG_POOL_N:
                        eng = "P"
                    elif r < DIAG_POOL_N + DIAG_ACT_N:
                        eng = "A"
                    else:
                        eng = "D"
                    state["nsel"] += 1
                    gi = state["nD"] % DIAG_GRP
                    if gi == 0:
                        gD = dgp.tile([P, DIAG_GRP, P], bft, tag="diag")
                        state["gD"] = gD
                    diag = state["gD"][:, gi, :]
                    state["nD"] += 1
                    if eng == "A":
                        nc.scalar.activation(
                            out=diag, in_=ident[:],
                            func=mybir.ActivationFunctionType.Identity,
                            scale=sc)
                    elif eng == "P":
                        nc.gpsimd.tensor_scalar(
                            out=diag, in0=ident[:], scalar1=sc,
                            scalar2=None, op0=mybir.AluOpType.mult)
                    else:
                        nc.vector.tensor_scalar(
                            out=diag, in0=ident[:], scalar1=sc,
                            scalar2=None, op0=mybir.AluOpType.mult)
                    aps.append(diag)
                diag_store[L] = aps

            # bootstrap: list 0's diags build in the startup window
            build_diags(0)

            ncopy = 0
            io, wo = 0, 0
            for ci in range(NCH):
                off, sz = CH_OFF[ci], CHUNKS[ci]
                nb = sz // 128
                out_ps = bigp.tile([COUT, CHUNKS[0]], f32, tag="big")
                for br in range(NBR):
                    for k in range(KK):
                        t = br * KK + k
                        last = (ci == NCH - 1 and br == NBR - 1
                                and k == KK - 1 and sz >= 1024)
                        tail2 = False
                        use8 = (FP8_TAPS and k in FP8_KSET and not last)
                        if last:
                            # split the final gather into separate tiles so
                            # earlier strips process while later parts
                            # transfer; asymmetric so only the final 512
                            # positions depend on the very last transfer
                            import os as _o
                            _gs = _o.environ.get("KERN_GSPLIT", "")
                            if _gs:
                                gsplit = [int(x) for x in _gs.split(",")]
                            elif sz == 2048:
                                gsplit = [1024, 512, 512]
                            else:
                                gsplit = [sz // 2, sz // 2]
                            ghalves = []
                            goff = 0
                            for gh, hsz in enumerate(gsplit):
                                gt = glp.tile([P, hsz // 128, 4 * C], bft,
                                              tag=f"glast{gh}")
                                nc.gpsimd.dma_gather(
                                    out_ap=gt[:, 0:hsz // 128, :],
                                    in_ap=src_ap,
                                    idxs_ap=idx_t[:, io + goff // 16:
                                                  io + (goff + hsz) // 16],
                                    num_idxs=hsz, num_idxs_reg=hsz,
                                    elem_size=4 * C, elem_step=2 * C,
                                    transpose=False, single_packet=False,
                                )
                                ghalves.append((goff // 128, gt))
                                goff += hsz
                            g = None
                        elif use8:
                            g = g8p.tile([P, CHUNKS[0] // 128, 4 * C],
                                         mybir.dt.float8e4, tag="g8")
                            nc.gpsimd.dma_gather(
                                out_ap=g[:, 0:nb, :], in_ap=src_ap8,
                                idxs_ap=idx_t[:, io:io + sz // 16],
                                num_idxs=sz, num_idxs_reg=sz,
                                elem_size=4 * C, elem_step=2 * C, transpose=False,
                                single_packet=False,
                            )
                        else:
                            g = gp.tile([P, CHUNKS[0] // 128, 4 * C], bft,
                                        tag="g")
                            nc.gpsimd.dma_gather(
                                out_ap=g[:, 0:nb, :], in_ap=src_ap,
                                idxs_ap=idx_t[:, io:io + sz // 16],
                                num_idxs=sz, num_idxs_reg=sz,
                                elem_size=4 * C, elem_step=2 * C, transpose=False,
                                single_packet=False,
                            )
                        sampT = sampp.tile([C, CHUNKS[0]], bft, tag="sampT")
                        if last:
                            ob_last = op.tile([COUT, CHUNKS[0]], bft,
                                              tag="outsb")
                        Lcur = ci * NBR * KK + br * KK + k
                        diags_cur = diag_store.pop(Lcur)
                        if last or tail2:
                            strips = [(i * 4, 4) for i in range(nb // 4)]
                        else:
                            strips = [(i * 8, min(8, nb - i * 8))
                                      for i in range((nb + 7) // 8)]
                        for qh, (qb0, qnb) in enumerate(strips):
                            tp = tpp.tile([C, 1024], f32, tag="tp")
                            for jb in range(qnb):
                                b = qb0 + jb
                                for c4 in range(4):
                                    diag = diags_cur[b * 4 + c4]
                                    if last:
                                        for gi_, gt_ in reversed(ghalves):
                                            if b >= gi_:
                                                gsrc = gt_[:, b - gi_, :]
                                                break
                                    else:
                                        gsrc = g[:, b, :]
                                    nc.tensor.matmul(
                                        out=tp[:, jb * P:(jb + 1) * P],
                                        lhsT=gsrc[:, c4 * C:(c4 + 1) * C],
                                        rhs=diag,
                                        start=(c4 == 0), stop=(c4 == 3),
                                    )
                            qoff = qb0 * P
                            dst = sampT[:, qoff:qoff + qnb * P]
                            ce = 1 if last else copy_engs[ncopy % len(copy_engs)]
                            ncopy += 1
                            if ce == 1:
                                nc.vector.tensor_copy(out=dst,
                                                      in_=tp[:, 0:qnb * P])
                            else:
                                nc.scalar.copy(out=dst, in_=tp[:, 0:qnb * P])
                            if last or tail2:
                                # deform per 512-strip so the drain pipelines
                                nc.tensor.matmul(
                                    out=out_ps[:, qoff:qoff + qnb * P],
                                    lhsT=w0_t[:, t, :],
                                    rhs=sampT[:, qoff:qoff + qnb * P],
                                    start=False, stop=last,
                                )
                                if last:
                                    nc.scalar.activation(
                                        out=ob_last[:, qoff:qoff + qnb * P],
                                        in_=out_ps[:, qoff:qoff + qnb * P],
                                        func=mybir.ActivationFunctionType.Identity,
                                        bias=bf_t[:], scale=1.0,
                                    )
                                    nc.sync.dma_start(
                                        out=out_d[:, off + qoff:
                                                  off + qoff + qnb * P],
                                        in_=ob_last[:, qoff:qoff + qnb * P])
                        if not (last or tail2):
                            for cs in range(0, sz, 512):
                                ce_ = min(cs + 512, sz)
                                nc.tensor.matmul(
                                    out=out_ps[:, cs:ce_],
                                    lhsT=w0_t[:, t, :],
                                    rhs=sampT[:, cs:ce_],
                                    start=(br == 0 and k == 0),
                                    stop=(br == NBR - 1 and k == KK - 1),
                                )
                        # build the NEXT list's diags one DMA period ahead
                        # of their consumers (emitted after this list's
                        # copies so ACT drains the tp chain first)
                        if Lcur + 1 < NLISTS_T:
                            build_diags(Lcur + 1)
                        io += sz // 16
                        wo += (sz // 128) * 4
                if ci == NCH - 1 and sz >= 1024:
                    pass  # drained per strip inside the last list's loop
                else:
                    out_sb = op.tile([COUT, CHUNKS[0]], bft, tag="outsb")
                    nc.scalar.activation(
                        out=out_sb[:, 0:sz], in_=out_ps[:, 0:sz],
                        func=mybir.ActivationFunctionType.Identity, bias=bf_t[:],
                        scale=1.0,
                    )
                    nc.sync.dma_start(out=out_d[:, off:off + sz],
                                      in_=out_sb[:, 0:sz])
    nc.finalize()
    return nc


def kernel(x, dm0, dm1, w0, w1, wf, bf):
    x = np.asarray(x, np.float32)
    dm0 = np.asarray(dm0, np.float32)
    dm1 = np.asarray(dm1, np.float32)
    w0 = np.asarray(w0, np.float32)
    w1 = np.asarray(w1, np.float32)
    wf = np.asarray(wf, np.float32)
    bfv = np.asarray(bf, np.float32)

    xPPp, xPP8, IDX, WS, W0T, BF = _host_precompute(x, dm0, dm1, w0, w1, wf, bfv)

    if "nc" not in _CACHE:
        _CACHE["nc"] = _build_nc()
    nc = _CACHE["nc"]

    in_maps = [
        {
            "xpp": np.ascontiguousarray(xPPp[i]),
            "xpp8": np.ascontiguousarray(xPP8[i]),
            "idx": np.ascontiguousarray(IDX[i].reshape(P, -1)),
            "ws": np.ascontiguousarray(WS[i].reshape(P, -1)),
            "w0t": W0T.reshape(C, -1),
            "bfv": BF,
        }
        for i in range(B)
    ]
    res = run_bass_kernel_spmd(nc, in_maps, core_ids=list(range(B)),
                               **_CACHE.get("run_kwargs", {}))
    _CACHE["last_results"] = res
    out = np.stack([np.asarray(res.results[i]["out"], np.float32)
                    for i in range(B)])
    return out.reshape(B, COUT, H, W)



# revision 36
# speedup vs baseline: 1.0562x; 1.0062x over previous
# Trainium2 Bass kernel for nn_DeformableInception (deformable conv x2 -> concat -> 1x1 conv).
#
# Sharding: data-parallel over batch B=8, one sample per NeuronCore (8 cores).
# Weights replicated. No collectives.
#
# Per-core device pipeline (per sample):
#   - x is stored in DRAM as parity-packed row pairs: slot (par, yy, xx) holds
#     image rows (2*yy+par, 2*yy+par+1) x 128ch bf16 (512B). A bilinear 2x2 patch
#     at (yb, xb) is two adjacent slots = ONE contiguous 1KB gather descriptor
#     (>=512B, so no DMA read-modify-write penalty).
#   - per (chunk, branch, tap): SWDGE dma_gather fetches one 1KB patch per output
#     position (positions land on partitions): g[pos, blk, 512] = [v00|v10|v01|v11].
#   - the bilinear blend runs on PE as "diagonal matmuls": for each corner,
#     matmul(out=tp[c, pos], lhsT=g_corner[pos, c], rhs=diag(w_corner)) accumulates
#     the weighted corner into PSUM. The diag tiles (identity * per-position folded
#     corner weight) are built by 4x-mode tensor_scalar on DVE (some on ACT), depend
#     only on host-precomputed weights (not the gather), and are allocated in
#     groups of DIAG_GRP per pool tile to amortize semaphore waits.
#   - tp (f32 PSUM) -> sampT (bf16 SBUF) on ACT, then one PSUM accumulator per
#     chunk takes all 18 taps of both branches: the 1x1 fuse conv and the concat
#     are folded into the per-tap weights on the host (W''_k = W_k @ Wf_br^T),
#     so only a bias-add (ACT activation) and the output DMA remain.
import sys

sys.path.insert(0, "/opt/trn_rl_repo")

import numpy as np
import ml_dtypes

import concourse.bass as bass
import concourse.mybir as mybir
from concourse.tile import TileContext
from concourse.masks import make_identity
from concourse import bacc
from concourse.bass_utils import run_bass_kernel_spmd

bf16 = ml_dtypes.bfloat16

# problem constants (hardcoded per spec)
B = 8
C = 128
H = W = 64
HW = H * W                 # 4096
COUT = 84
K = 3
PAD = 1
KK = K * K                 # 9
NBR = 2                    # two deformable branches
# position chunks; small enough that two PSUM accumulators fit (overlapped
# drain) and the final pipeline drain is short, big enough that gather
# descriptor-prep on Pool stays ahead of the DMA transfers
import os as _osmod
_chunks_env = _osmod.environ.get("KERN_CHUNKS", "2048,2048")
CHUNKS = [int(t) for t in _chunks_env.split(",")]
NCH = len(CHUNKS)
CH_OFF = [sum(CHUNKS[:i]) for i in range(NCH)]
NLISTS = NBR * KK * NCH    # gather lists (one per chunk x branch x tap)
IDXCOLS = HW // 16         # idx cols per (br, tap) across all chunks
WSCOLS = (HW // 128) * 4   # ws cols per (br, tap) across all chunks
NSLOT = 2 * 32 * 64        # 4096 parity-packed patch slots

P = 128
f32 = mybir.dt.float32
bft = mybir.dt.bfloat16
i16 = mybir.dt.int16

import os as _os
# diag builds rotate across DVE / ACT / Pool in a cycle of DIAG_CYCLE:
# first DIAG_POOL_N of each cycle go to Pool (gpsimd), next DIAG_ACT_N to
# ACT, rest to DVE. Pool diag ~273ns, ACT ~292ns, DVE ~94ns in the cost
# model; the split keeps every engine under the DMA-transfer bound.
DIAG_CYCLE = int(_os.environ.get("KERN_DIAG_CYCLE", "16"))
DIAG_POOL_N = int(_os.environ.get("KERN_DIAG_POOL_N", "0"))
DIAG_ACT_N = int(_os.environ.get("KERN_DIAG_ACT_N", "2"))
# for the last TAIL_DVE lists of the run, put ALL runtime diags on DVE —
# ACT (3.1x slower per diag) stays saturated to the end and paces the
# drain, while DVE has slack there
TAIL_DVE = int(_os.environ.get("KERN_TAIL_DVE", "8"))
# within the tail, send every TAIL_POOL_N-of-4 diags to Pool (its SEQ is
# free of gather waits once the last gathers have issued)
TAIL_POOL_N = int(_os.environ.get("KERN_TAIL_POOL_N", "0"))
# number of trailing blocks per list whose diags build on ACT (0 = use
# the DIAG_CYCLE rotation instead)
ACT_BLK = int(_os.environ.get("KERN_ACT_BLK", "0"))
# whether tail lists also use the ACT block span
TAIL_ACT = int(_os.environ.get("KERN_TAIL_ACT", "0"))
# tp->sampT copy engine: 1=DVE, 2=ACT (per-copy round robin list)
COPY_ENGS = _os.environ.get("KERN_COPY_ENGS", "2")
GP_BUFS = int(_os.environ.get("KERN_GP_BUFS", "4"))
TPP_BUFS = int(_os.environ.get("KERN_TPP_BUFS", "2"))
SAMP_BUFS = int(_os.environ.get("KERN_SAMP_BUFS", "3"))
DIAG_BUFS = int(_os.environ.get("KERN_DIAG_BUFS", "10"))
DIAG_A_BUFS = int(_os.environ.get("KERN_DIAG_A_BUFS", "3"))
DIAG_GRP = int(_os.environ.get("KERN_DIAG_GRP", "16"))  # diags per pool tile
# PSUM budget: out_ps (f32, CHUNKS[0] wide) banks * bufs + tp 2*TPP_BUFS banks
# must fit 8 banks; double-buffer the accumulator only for chunks <= 1024
PREBUILD_LISTS = int(_os.environ.get("KERN_PREBUILD_LISTS", "0"))
FP8_TAPS = int(_os.environ.get("KERN_FP8_TAPS", "1"))
G8_BUFS = int(_os.environ.get("KERN_G8_BUFS", "2"))
POOL_BUILDS = set(int(t) for t in _os.environ.get(
    "KERN_POOL_BUILDS", "").split(",") if t != "")
ACT_BUILDS = set(int(t) for t in _os.environ.get(
    "KERN_ACT_BUILDS", "7,12").split(",") if t != "")
FP8_KSET = set(int(t) for t in
               _os.environ.get("KERN_FP8_KSET", "1,4").split(","))
BIGP_BUFS = int(_os.environ.get("KERN_BIGP_BUFS", "0"))
if BIGP_BUFS == 0:
    BIGP_BUFS = 2 if CHUNKS[0] <= 1024 else 1

_CACHE = {}


def _host_precompute(x, dm0, dm1, w0, w1, wf, bfv):
    """Numpy precompute: patch-slot gather indices + 2D-folded bilinear corner
    weights, parity-packed x, weight repacks."""
    ky = np.repeat(np.arange(K) - PAD, K).astype(np.float32)
    kx = np.tile(np.arange(K) - PAD, K).astype(np.float32)
    base_y = np.arange(H, dtype=np.float32).reshape(1, 1, H, 1)
    base_x = np.arange(W, dtype=np.float32).reshape(1, 1, 1, W)

    idx_all = np.zeros((B, NBR, KK, HW), np.int16)        # patch slot per (tap,pos)
    w_all = np.zeros((B, NBR, KK, 4, HW), np.float32)     # r0c0,r1c0,r0c1,r1c1

    for br, dm in ((0, dm0), (1, dm1)):
        off = dm.reshape(B, KK, 2, H, W)
        py = off[:, :, 0] + base_y + ky.reshape(1, KK, 1, 1)
        px = off[:, :, 1] + base_x + kx.reshape(1, KK, 1, 1)
        y0 = np.floor(py); x0 = np.floor(px)
        wy1 = py - y0; wx1 = px - x0
        wy0 = 1.0 - wy1; wx0 = 1.0 - wx1
        y0i = y0.astype(np.int64); x0i = x0.astype(np.int64)
        yb = np.clip(y0i, 0, H - 2)
        xb = np.clip(x0i, 0, W - 2)
        slot = (yb & 1) * (32 * 64) + (yb >> 1) * 64 + xb
        idx_all[:, br] = slot.reshape(B, KK, HW).astype(np.int16)
        w4 = np.zeros((2, 2) + py.shape, np.float32)      # [rp, cp, B, KK, H, W]
        for r, wy in ((0, wy0), (1, wy1)):
            yi = y0i + r
            rv = ((yi >= 0) & (yi < H)).astype(np.float32)
            rp = np.clip(yi, 0, H - 1) - yb               # 0 or 1
            for c, wx in ((0, wx0), (1, wx1)):
                xi = x0i + c
                cv = ((xi >= 0) & (xi < W)).astype(np.float32)
                cp = np.clip(xi, 0, W - 1) - xb
                contrib = wy * wx * rv * cv
                for rr in (0, 1):
                    for cc in (0, 1):
                        w4[rr, cc] += np.where((rp == rr) & (cp == cc), contrib, 0.0)
        # corner order matches patch byte layout [v00, v10, v01, v11]
        w_all[:, br, :, 0] = w4[0, 0].reshape(B, KK, HW)
        w_all[:, br, :, 1] = w4[1, 0].reshape(B, KK, HW)
        w_all[:, br, :, 2] = w4[0, 1].reshape(B, KK, HW)
        w_all[:, br, :, 3] = w4[1, 1].reshape(B, KK, HW)

    # xPP [B, NSLOT+2, 2C] bf16: slot (par, yy, xx) = rows (2yy+par, 2yy+par+1)
    xhwc = np.transpose(x, (0, 2, 3, 1))                  # [B, H, W, C]
    xPP = np.zeros((B, 2, 32, 64, 2, C), np.float32)
    for par in (0, 1):
        for rp in (0, 1):
            start = par + rp
            rows = xhwc[:, start::2, :, :]                # [B, n, W, C]
            n = min(rows.shape[1], 32)
            xPP[:, par, :n, :, rp, :] = rows[:, :n]
    xPP = xPP.reshape(B, NSLOT, 2 * C)
    xPPf = np.concatenate([xPP, np.zeros((B, 2, 2 * C), np.float32)], axis=1)
    xPPp = xPPf.astype(bf16)
    xPP8 = xPPf.astype(ml_dtypes.float8_e4m3fn)

    # IDX [B, 128, totalcols] int16: lists in consumption order (chunk, br, k),
    # each wrapped (j%16, j//16), replicated across the 8 gpsimd cores'
    # 16-partition groups, then concatenated along cols so one DMA loads it.
    # WS [B, 128, totalwcols] f32 likewise (per block: 4 corner weights).
    idx_cols = []
    ws_cols = []
    for ci in range(NCH):
        off, sz = CH_OFF[ci], CHUNKS[ci]
        for br in range(NBR):
            for k in range(KK):
                seq = idx_all[:, br, k, off:off + sz]     # [B, sz]
                wr = np.transpose(seq.reshape(B, sz // 16, 16), (0, 2, 1))
                wr = np.broadcast_to(wr[:, None, :, :], (B, 8, 16, sz // 16))
                idx_cols.append(wr.reshape(B, P, sz // 16))
                wsl = w_all[:, br, k, :, off:off + sz]    # [B, 4, sz]
                wsl = wsl.reshape(B, 4, sz // 128, P)
                wsl = np.transpose(wsl, (0, 3, 2, 1))     # [B, p, blk, c4]
                ws_cols.append(wsl.reshape(B, P, (sz // 128) * 4))
    IDX = np.ascontiguousarray(np.concatenate(idx_cols, axis=2))
    WS = np.ascontiguousarray(np.concatenate(ws_cols, axis=2)).astype(bf16)

    # W0T [128, NTAPS*COUT] bf16: lhsT per (branch, tap) with the 1x1 fuse conv
    # folded in (W''_k = W_k @ Wf_br^T), device layout
    WFT = [wf[:, :COUT, 0, 0].T, wf[:, COUT:, 0, 0].T]    # [84in, 84out] per br
    W0T = np.zeros((NBR * KK, C, COUT), np.float32)
    for br, w in ((0, w0), (1, w1)):
        for k in range(KK):
            W0T[br * KK + k] = w[:, :, k // K, k % K].T @ WFT[br]
    W0T = np.ascontiguousarray(np.transpose(W0T, (1, 0, 2))).astype(bf16)

    BF = bfv.reshape(COUT, 1).astype(np.float32)
    return xPPp, xPP8, IDX, WS, W0T, BF


def _build_nc():
    nc = bacc.Bacc()
    xpp_d = nc.declare_dram_parameter("xpp", [NSLOT + 2, 2 * C], bft, isOutput=False)
    xpp8_d = nc.declare_dram_parameter("xpp8", [NSLOT + 2, 2 * C],
                                       mybir.dt.float8e4, isOutput=False)
    idx_d = nc.declare_dram_parameter("idx", [P, NBR * KK * IDXCOLS], i16, isOutput=False)
    ws_d = nc.declare_dram_parameter("ws", [P, NBR * KK * WSCOLS], bft, isOutput=False)
    w0_d = nc.declare_dram_parameter("w0t", [C, NBR * KK * COUT], bft, isOutput=False)
    bf_d = nc.declare_dram_parameter("bfv", [COUT, 1], f32, isOutput=False)
    out_d = nc.declare_dram_parameter("out", [COUT, HW], bft, isOutput=True)

    # patch gather source: elem i = slot i (256 elems), read 512 elems (2 slots)
    src_ap = bass.AP(tensor=xpp_d, offset=0, ap=[[2 * C, NSLOT], [1, 4 * C]])
    src_ap8 = bass.AP(tensor=xpp8_d, offset=0, ap=[[2 * C, NSLOT], [1, 4 * C]])

    copy_engs = [int(t) for t in COPY_ENGS.split(",")]

    with TileContext(nc) as tc:
        with tc.tile_pool(name="const", bufs=1) as const, \
             tc.tile_pool(name="gp", bufs=GP_BUFS) as gp, \
             tc.tile_pool(name="glp", bufs=1) as glp, \
             tc.tile_pool(name="g8p", bufs=G8_BUFS) as g8p, \
             tc.tile_pool(name="dgp", bufs=DIAG_BUFS) as dgp, \
             tc.tile_pool(name="dgpa", bufs=DIAG_A_BUFS) as dgpa, \
             tc.tile_pool(name="sampp", bufs=SAMP_BUFS) as sampp, \
             tc.tile_pool(name="op", bufs=2) as op, \
             tc.tile_pool(name="tpp", bufs=TPP_BUFS, space="PSUM") as tpp, \
             tc.tile_pool(name="bigp", bufs=BIGP_BUFS, space="PSUM") as bigp:
            TIC = NBR * KK * IDXCOLS
            TWC = NBR * KK * WSCOLS
            # split the idx/ws loads so the first gathers launch ASAP;
            # issue them before any compute setup so the DMA queue drains
            # while identity/ws-convert run
            NIH = 2 * (CHUNKS[0] // 16)
            NWH = 2 * ((CHUNKS[0] // 128) * 4)
            idx_t = const.tile([P, TIC], i16)
            nc.sync.dma_start(out=idx_t[:, 0:NIH // 2], in_=idx_d[:, 0:NIH // 2])
            ws_b = const.tile([P, TWC], bft)
            nc.sync.dma_start(out=ws_b[:, 0:NWH], in_=ws_d[:, 0:NWH])
            nc.sync.dma_start(out=idx_t[:, NIH // 2:NIH],
                              in_=idx_d[:, NIH // 2:NIH])
            ident = const.tile([P, P], bft)
            make_identity(nc, ident[:])
            ws_t = const.tile([P, TWC], f32)
            nc.scalar.copy(out=ws_t[:, 0:NWH], in_=ws_b[:, 0:NWH])
            # remainder const loads: small pieces emitted after w0t so their
            # arrival on the serial DMA resource trails the first gather's
            # descriptor chain instead of blocking it (consumers only need
            # this data a few lists in)
            w0_t = const.tile([C, NBR * KK, COUT], bft)
            nhalf = NBR * KK // 2
            nc.sync.dma_start(out=w0_t[:, 0:nhalf, :],
                              in_=w0_d[:, 0:nhalf * COUT])
            nc.sync.dma_start(out=w0_t[:, nhalf:, :],
                              in_=w0_d[:, nhalf * COUT:])
            bf_t = const.tile([COUT, 1], f32)
            nc.sync.dma_start(out=bf_t[:], in_=bf_d[:])
            NSL = int(_os.environ.get("KERN_CONST_SLICES", "1"))
            rem = TIC - NIH
            remw = TWC - NWH
            for s_ in range(NSL):
                a_ = NIH + (rem * s_) // NSL
                b_ = NIH + (rem * (s_ + 1)) // NSL
                nc.sync.dma_start(out=idx_t[:, a_:b_], in_=idx_d[:, a_:b_])
                aw = NWH + (remw * s_) // NSL
                bw = NWH + (remw * (s_ + 1)) // NSL
                nc.sync.dma_start(out=ws_b[:, aw:bw], in_=ws_d[:, aw:bw])
            nc.scalar.copy(out=ws_t[:, NWH:], in_=ws_b[:, NWH:])

            # --- software-pipelined diag builds -------------------------
            # Diags depend only on ws, so build list L's diags during the
            # processing of list L-1 (one full DMA period of slack). This
            # removes diag-build latency from the per-list critical chain:
            # blends never wait on a diag that was emitted just before them.
            NLISTS_T = NCH * NBR * KK
            wo_of = [0]
            for ci_ in range(NCH):
                for _ in range(NBR * KK):
                    wo_of.append(wo_of[-1] + (CHUNKS[ci_] // 128) * 4)
            diag_store = {}  # L -> list of 64 diag APs
            # per-engine group tiles so one engine's SEQ never blocks on a
            # buffer whose writers/consumers belong to the other engine
            state = {"nD": 0, "gD": None, "nA": 0, "gA": None, "nsel": 0}

            def build_diags(L):
                nd = (CHUNKS[L // (NBR * KK)] // 128) * 4
                wo_ = wo_of[L]
                aps = []
                for di in range(nd):
                    sc = ws_t[:, wo_ + di:wo_ + di + 1]
                    # engine selection: ACT_BLK>0 sends the last ACT_BLK
                    # blocks' diags to ACT; otherwise rotate every cycle of
                    # DIAG_CYCLE (first DIAG_POOL_N to Pool, next DIAG_ACT_N
                    # to ACT, rest DVE). Tail lists force DVE.
                    r = state["nsel"] % DIAG_CYCLE
                    if L >= NLISTS_T - TAIL_DVE and not TAIL_ACT:
                        eng = "D"
                    elif ACT_BLK > 0:
                        eng = "A" if di >= nd - 4 * ACT_BLK else "D"
                    elif r < DIAG_POOL_N:
                        eng = "P"
                    elif r < DIAG_POOL_N + DIAG_ACT_N:
                        eng = "A"
                    else:
                        eng = "D"
                    state["nsel"] += 1
                    gi = state["nD"] % DIAG_GRP
                    if gi == 0:
                        gD = dgp.tile([P, DIAG_GRP, P], bft, tag="diag")
                        state["gD"] = gD
                    diag = state["gD"][:, gi, :]
                    state["nD"] += 1
                    if eng == "A":
                        nc.scalar.activation(
                            out=diag, in_=ident[:],
                            func=mybir.ActivationFunctionType.Identity,
                            scale=sc)
                    elif eng == "P":
                        nc.gpsimd.tensor_scalar(
                            out=diag, in0=ident[:], scalar1=sc,
                            scalar2=None, op0=mybir.AluOpType.mult)
                    else:
                        nc.vector.tensor_scalar(
                            out=diag, in0=ident[:], scalar1=sc,
                            scalar2=None, op0=mybir.AluOpType.mult)
                    aps.append(diag)
                diag_store[L] = aps

            # bootstrap: list 0's diags build in the startup window
            build_diags(0)

            # prebuild the final PREBUILD_LISTS lists' diags into a const
            # tile during the early slack (they depend only on ws), so the
            # drain isn't gated on just-in-time DVE diag builds
            if PREBUILD_LISTS > 0:
                npre = (CHUNKS[-1] // 128) * 4 * PREBUILD_LISTS
                dlast = const.tile([P, npre, P], bft)
                for j_ in range(PREBUILD_LISTS):
                    Lp = NLISTS_T - PREBUILD_LISTS + j_
                    nd_ = (CHUNKS[Lp // (NBR * KK)] // 128) * 4
                    wo_ = wo_of[Lp]
                    aps = []
                    for di in range(nd_):
                        slot = j_ * (CHUNKS[-1] // 128) * 4 + di
                        nc.vector.tensor_scalar(
                            out=dlast[:, slot, :], in0=ident[:],
                            scalar1=ws_t[:, wo_ + di:wo_ + di + 1],
                            scalar2=None, op0=mybir.AluOpType.mult)
                        aps.append(dlast[:, slot, :])
                    diag_store[Lp] = aps

            ncopy = 0
            io, wo = 0, 0
            for ci in range(NCH):
                off, sz = CH_OFF[ci], CHUNKS[ci]
                nb = sz // 128
                out_ps = bigp.tile([COUT, CHUNKS[0]], f32, tag="big")
                for br in range(NBR):
                    for k in range(KK):
                        t = br * KK + k
                        last = (ci == NCH - 1 and br == NBR - 1
                                and k == KK - 1 and sz >= 1024)
                        tail2 = False
                        use8 = (FP8_TAPS and k in FP8_KSET and not last)
                        if last:
                            # split the final gather into separate tiles so
                            # earlier strips process while later parts
                            # transfer; asymmetric so only the final 512
                            # positions depend on the very last transfer
                            import os as _o
                            _gs = _o.environ.get("KERN_GSPLIT", "")
                            if _gs:
                                gsplit = [int(x) for x in _gs.split(",")]
                            elif sz == 2048:
                                gsplit = [1024, 512, 512]
                            else:
                                gsplit = [sz // 2, sz // 2]
                            ghalves = []
                            goff = 0
                            for gh, hsz in enumerate(gsplit):
                                gt = glp.tile([P, hsz // 128, 4 * C], bft,
                                              tag=f"glast{gh}")
                                nc.gpsimd.dma_gather(
                                    out_ap=gt[:, 0:hsz // 128, :],
                                    in_ap=src_ap,
                                    idxs_ap=idx_t[:, io + goff // 16:
                                                  io + (goff + hsz) // 16],
                                    num_idxs=hsz, num_idxs_reg=hsz,
                                    elem_size=4 * C, elem_step=2 * C,
                                    transpose=False, single_packet=False,
                                )
                                ghalves.append((goff // 128, gt))
                                goff += hsz
                            g = None
                        elif use8:
                            g = g8p.tile([P, CHUNKS[0] // 128, 4 * C],
                                         mybir.dt.float8e4, tag="g8")
                            nc.gpsimd.dma_gather(
                                out_ap=g[:, 0:nb, :], in_ap=src_ap8,
                                idxs_ap=idx_t[:, io:io + sz // 16],
                                num_idxs=sz, num_idxs_reg=sz,
                                elem_size=4 * C, elem_step=2 * C, transpose=False,
                                single_packet=False,
                            )
                        else:
                            g = gp.tile([P, CHUNKS[0] // 128, 4 * C], bft,
                                        tag="g")
                            nc.gpsimd.dma_gather(
                                out_ap=g[:, 0:nb, :], in_ap=src_ap,
                                idxs_ap=idx_t[:, io:io + sz // 16],
                                num_idxs=sz, num_idxs_reg=sz,
                                elem_size=4 * C, elem_step=2 * C, transpose=False,
                                single_packet=False,
                            )
                        sampT = sampp.tile([C, CHUNKS[0]], bft, tag="sampT")
                        if last:
                            ob_last = op.tile([COUT, CHUNKS[0]], bft,
                                              tag="outsb")
                        Lcur = ci * NBR * KK + br * KK + k
                        diags_cur = diag_store.pop(Lcur)
                        if last or tail2:
                            strips = [(i * 4, 4) for i in range(nb // 4)]
                        else:
                            strips = [(i * 8, min(8, nb - i * 8))
                                      for i in range((nb + 7) // 8)]
                        for qh, (qb0, qnb) in enumerate(strips):
                            tp = tpp.tile([C, 1024], f32, tag="tp")
                            for jb in range(qnb):
                                b = qb0 + jb
                                for c4 in range(4):
                                    diag = diags_cur[b * 4 + c4]
                                    if last:
                                        for gi_, gt_ in reversed(ghalves):
                                            if b >= gi_:
                                                gsrc = gt_[:, b - gi_, :]
                                                break
                                    else:
                                        gsrc = g[:, b, :]
                                    nc.tensor.matmul(
                                        out=tp[:, jb * P:(jb + 1) * P],
                                        lhsT=gsrc[:, c4 * C:(c4 + 1) * C],
                                        rhs=diag,
                                        start=(c4 == 0), stop=(c4 == 3),
                                    )
                            qoff = qb0 * P
                            dst = sampT[:, qoff:qoff + qnb * P]
                            ce = (int(_os.environ.get("KERN_LAST_COPY", "1"))
                                  if last else copy_engs[ncopy % len(copy_engs)])
                            ncopy += 1
                            if ce == 1:
                                nc.vector.tensor_copy(out=dst,
                                                      in_=tp[:, 0:qnb * P])
                            elif ce == 3:
                                nc.gpsimd.tensor_copy(out=dst,
                                                      in_=tp[:, 0:qnb * P])
                            else:
                                nc.scalar.copy(out=dst, in_=tp[:, 0:qnb * P])
                            if last or tail2:
                                # deform per 512-strip so the drain pipelines
                                nc.tensor.matmul(
                                    out=out_ps[:, qoff:qoff + qnb * P],
                                    lhsT=w0_t[:, t, :],
                                    rhs=sampT[:, qoff:qoff + qnb * P],
                                    start=False, stop=last,
                                )
                                if last:
                                    nc.scalar.activation(
                                        out=ob_last[:, qoff:qoff + qnb * P],
                                        in_=out_ps[:, qoff:qoff + qnb * P],
                                        func=mybir.ActivationFunctionType.Identity,
                                        bias=bf_t[:], scale=1.0,
                                    )
                                    nc.sync.dma_start(
                                        out=out_d[:, off + qoff:
                                                  off + qoff + qnb * P],
                                        in_=ob_last[:, qoff:qoff + qnb * P])
                        if not (last or tail2):
                            for cs in range(0, sz, 512):
                                ce_ = min(cs + 512, sz)
                                nc.tensor.matmul(
                                    out=out_ps[:, cs:ce_],
                                    lhsT=w0_t[:, t, :],
                                    rhs=sampT[:, cs:ce_],
                                    start=(br == 0 and k == 0),
                                    stop=(br == NBR - 1 and k == KK - 1),
                                )
                        # build the NEXT list's diags one DMA period ahead
                        # of their consumers (emitted after this list's
                        # copies so ACT drains the tp chain first);
                        # prebuilt tail lists are already in diag_store
                        if (Lcur + 1 < NLISTS_T
                                and Lcur + 1 not in diag_store
                                and Lcur + 1 < NLISTS_T - PREBUILD_LISTS):
                            build_diags(Lcur + 1)
                        io += sz // 16
                        wo += (sz // 128) * 4
                if ci == NCH - 1 and sz >= 1024:
                    pass  # drained per strip inside the last list's loop
                else:
                    out_sb = op.tile([COUT, CHUNKS[0]], bft, tag="outsb")
                    nc.scalar.activation(
                        out=out_sb[:, 0:sz], in_=out_ps[:, 0:sz],
                        func=mybir.ActivationFunctionType.Identity, bias=bf_t[:],
                        scale=1.0,
                    )
                    nc.sync.dma_start(out=out_d[:, off:off + sz],
                                      in_=out_sb[:, 0:sz])
    nc.finalize()
    return nc


def kernel(x, dm0, dm1, w0, w1, wf, bf):
    x = np.asarray(x, np.float32)
    dm0 = np.asarray(dm0, np.float32)
    dm1 = np.asarray(dm1, np.float32)
    w0 = np.asarray(w0, np.float32)
    w1 = np.asarray(w1, np.float32)
    wf = np.asarray(wf, np.float32)
    bfv = np.asarray(bf, np.float32)

    xPPp, xPP8, IDX, WS, W0T, BF = _host_precompute(x, dm0, dm1, w0, w1, wf, bfv)

    if "nc" not in _CACHE:
        _CACHE["nc"] = _build_nc()
    nc = _CACHE["nc"]

    in_maps = [
        {
            "xpp": np.ascontiguousarray(xPPp[i]),
            "xpp8": np.ascontiguousarray(xPP8[i]),
            "idx": np.ascontiguousarray(IDX[i].reshape(P, -1)),
            "ws": np.ascontiguousarray(WS[i].reshape(P, -1)),
            "w0t": W0T.reshape(C, -1),
            "bfv": BF,
        }
        for i in range(B)
    ]
    res = run_bass_kernel_spmd(nc, in_maps, core_ids=list(range(B)),
                               **_CACHE.get("run_kwargs", {}))
    _CACHE["last_results"] = res
    out = np.stack([np.asarray(res.results[i]["out"], np.float32)
                    for i in range(B)])
    return out.reshape(B, COUT, H, W)

